# revision 17
# baseline (speedup 1.0000x reference)
"""Depthwise 3x3 conv (single shared 2D kernel), pad=1 stride=1.

X: (16, 64, 256, 256) f32, K: (3, 3) f32  ->  out same shape as X.

Strategy: data-parallel over the 8 NeuronCores; each core gets 128 of the
1024 (B*C) independent 256x256 images.

Per-core compute: express the H-direction 3-tap conv as a banded-matrix
matmul on the TensorEngine (contraction over the partition axis), and the
W-direction taps as free-axis offsets of the rhs access pattern, so one
PSUM bank accumulates all 9 taps in 3 matmuls. An image is split into two
128-row halves living side by side in the free axis ([128, 2, 256] tiles).
The two output rows that straddle the half boundary (127, 128) can't be
produced by a 128-partition contraction, so they are computed for all 128
images at once in a small batched DVE pass (partition axis = image index).

float32r is used for the matmuls (full-rate PE streaming vs 4x-slow fp32).
"""

import numpy as np

B, C, H, W = 16, 64, 256, 256
NCORES = 8
NIMG = (B * C) // NCORES  # 128 images per core
HH = H // 2  # 128

_PROGRAM = None


def _build_program():
    import concourse.bacc as bacc
    import concourse.mybir as mybir
    import concourse.tile as tile

    f32 = mybir.dt.float32
    f32r = mybir.dt.float32r
    mult = mybir.AluOpType.mult
    add = mybir.AluOpType.add

    nc = bacc.Bacc(
        "TRN2", target_bir_lowering=False, debug=False, num_devices=NCORES
    )
    # x/a are float32r (TF32) end-to-end: the BIR verifier requires every
    # producer feeding an FP32r matmult to itself be FP32r-typed.
    x = nc.declare_dram_parameter("x", [NIMG, H, W], f32r, isOutput=False)
    a = nc.declare_dram_parameter("a", [3, 128, 128], f32r, isOutput=False)
    # rows 126..129 of every image, host-sliced, plain f32 for the DVE pass
    xg = nc.declare_dram_parameter("xg", [NIMG, 4, W], f32, isOutput=False)
    kr = nc.declare_dram_parameter("kr", [128, 9], f32, isOutput=False)
    y = nc.declare_dram_parameter("y", [NIMG, H, W], f32, isOutput=True)

    with tile.TileContext(nc) as tc:
        with (
            tc.tile_pool(name="wts", bufs=1) as wts,
            tc.tile_pool(name="xin", bufs=4) as xin,
            tc.tile_pool(name="oub", bufs=4) as oub,
            tc.tile_pool(name="bnd", bufs=1) as bnd,
            tc.tile_pool(name="ps", bufs=4, space="PSUM") as ps,
            tc.tile_pool(name="pswarm", bufs=1, space="PSUM") as pswarm,
        ):
            # banded lhsT weights, one [128, 128] matrix per kernel column dj
            a_sb = wts.tile([128, 3, 128], f32r)
            nc.sync.dma_start(out=a_sb[:], in_=a.transpose([1, 0, 2]))
            kr_sb = wts.tile([128, 9], f32)
            nc.sync.dma_start(out=kr_sb[:], in_=kr[:])

            ar = a_sb[:]

            # Warm-up matmul so the PE observes a_sb's DMA tick here; the
            # image-loop matmuls then only wait on their own xt DMA (the
            # fused f32r weight-load instruction has few sync-wait slots).
            warm_ps = pswarm.tile([128, 2], f32)
            nc.tensor.matmul(
                warm_ps[:], ar[:, 1, :], ar[:, 1, 0:2], start=True, stop=True
            )

            # f32r matmuls stream element PAIRS: every free-dim count must be
            # even and the PSUM dst must start 8B-aligned at partition 0. So
            # keep the W-halo as zeroed pad columns (258-wide tiles) and make
            # all three matmuls full-width with rhs offset dj. Ring of NBUF
            # manually-reused tiles so the pads are memset only once.
            NBUF = 4
            xts = []
            for b in range(NBUF):
                xtb = xin.tile([128, 2, W + 2], f32r, name=f"xtb{b}")
                nc.gpsimd.memset(xtb[:].bitcast(f32), 0.0)
                xts.append(xtb)

            for i in range(NIMG):
                xt = xts[i % NBUF]
                # partition p holds image rows p (half 0) and p+128 (half 1)
                nc.sync.dma_start(
                    out=xt[:, :, 1 : W + 1],
                    in_=x[i].rearrange("(t p) w -> p t w", t=2),
                )
                pt = ps.tile([128, 2, W], f32)
                # out[m, h, w] += sum_h' A_dj[h', m] * X[h', h, w + dj - 1]
                for dj in range(3):
                    nc.tensor.matmul(
                        pt[:, :, :],
                        ar[:, dj, :],
                        xt[:, :, dj : dj + W],
                        start=(dj == 0),
                        stop=(dj == 2),
                    )
                ot = oub.tile([128, 2, W], f32)
                nc.vector.tensor_copy(ot[:], pt[:])
                # rows 0..126 from half 0; rows 129..255 from half 1
                nc.sync.dma_start(out=y[i, 0 : HH - 1, :], in_=ot[0 : HH - 1, 0, :])
                nc.sync.dma_start(out=y[i, HH + 1 : H, :], in_=ot[1:128, 1, :])

            # batched pass for output rows 127/128 of every image:
            # partition axis = image index, 9-tap MAC on the DVE.
            g = bnd.tile([128, 4, W], f32)
            nc.sync.dma_start(out=g[:], in_=xg[:])
            yb = bnd.tile([128, 2, W], f32)
            for r in range(2):  # output image rows 127 (r=0) and 128 (r=1)
                out_row = yb[:, r, :]
                taps = [(0, 1)] + [
                    (di, dj) for di in range(3) for dj in range(3) if (di, dj) != (0, 1)
                ]
                for di, dj in taps:
                    kap = kr_sb[:, 3 * di + dj : 3 * di + dj + 1]
                    if dj == 1:
                        o_sl, i_sl = out_row[:, 0:W], g[:, r + di, 0:W]
                    elif dj == 0:
                        o_sl, i_sl = out_row[:, 1:W], g[:, r + di, 0 : W - 1]
                    else:
                        o_sl, i_sl = out_row[:, 0 : W - 1], g[:, r + di, 1:W]
                    if (di, dj) == (0, 1):
                        nc.vector.tensor_scalar_mul(o_sl, i_sl, kap)
                    else:
                        nc.vector.scalar_tensor_tensor(
                            o_sl, i_sl, kap, o_sl, mult, add
                        )
            nc.sync.dma_start(out=y[:, HH - 1 : HH + 1, :], in_=yb[:])

    nc.compile()
    return nc


def _get_program():
    global _PROGRAM
    if _PROGRAM is None:
        _PROGRAM = _build_program()
    return _PROGRAM


def _banded(K):
    # A[dj][h', m] = K[h'-m+1, dj] for h' in {m-1, m, m+1} within [0, 128)
    A = np.zeros((3, 128, 128), np.float32)
    for dj in range(3):
        for di in range(3):
            v = K[di, dj]
            for m in range(128):
                hp = m + di - 1
                if 0 <= hp < 128:
                    A[dj, hp, m] = v
    return A


def _run(X, K, trace=False, trace_kwargs=None):
    from concourse.bass_utils import run_bass_kernel_spmd

    X = np.ascontiguousarray(np.asarray(X), dtype=np.float32)
    K = np.ascontiguousarray(np.asarray(K), dtype=np.float32)
    assert X.shape == (B, C, H, W) and K.shape == (3, 3)

    nc = _get_program()
    A = _banded(K)
    kr = np.ascontiguousarray(np.tile(K.reshape(1, 9), (128, 1)), np.float32)
    Xf = X.reshape(B * C, H, W)
    in_maps = [
        {
            "x": np.ascontiguousarray(Xf[c * NIMG : (c + 1) * NIMG]),
            "a": A,
            "xg": np.ascontiguousarray(
                Xf[c * NIMG : (c + 1) * NIMG, HH - 2 : HH + 2, :]
            ),
            "kr": kr,
        }
        for c in range(NCORES)
    ]
    kw = {}
    if trace:
        kw["trace"] = True
        if trace_kwargs:
            kw.update(trace_kwargs)
    res = run_bass_kernel_spmd(nc, in_maps, list(range(NCORES)), **kw)
    out = np.stack([res.results[c]["y"] for c in range(NCORES)])
    return out.reshape(B, C, H, W), res


def kernel(X, K):
    out, _ = _run(X, K)
    return out


def _bench(X, K, chain_lens=(1, 16), reps=4):
    """Estimate per-execution HW time: run `chain` sequential kernel
    executions inside one jitted shard_map call (each exec's output buffers
    feed the next), and take the wall-clock slope between chain lengths to
    cancel dispatch / RPC constants."""
    import time as _time

    import jax
    import numpy as jnp_np
    from jax.sharding import Mesh, PartitionSpec
    from jax.experimental.shard_map import shard_map

    from concourse import bass2jax

    bass2jax.install_neuronx_cc_hook()

    X = np.ascontiguousarray(np.asarray(X), dtype=np.float32)
    K = np.ascontiguousarray(np.asarray(K), dtype=np.float32)
    nc = _get_program()
    A = _banded(K)
    kr = np.ascontiguousarray(np.tile(K.reshape(1, 9), (128, 1)), np.float32)
    Xf = X.reshape(B * C, H, W)

    import concourse.mybir as mybir

    partition_name = (
        nc.partition_id_tensor.name if nc.partition_id_tensor else None
    )
    in_names, out_names, out_avals = [], [], []
    for alloc in nc.m.functions[0].allocations:
        if not isinstance(alloc, mybir.MemoryLocationSet):
            continue
        name = alloc.memorylocations[0].name
        if alloc.kind == "ExternalInput":
            if name != partition_name:
                in_names.append(name)
        elif alloc.kind == "ExternalOutput":
            out_names.append(name)
            out_avals.append(
                jax.core.ShapedArray(
                    tuple(alloc.tensor_shape), mybir.dt.np(alloc.dtype)
                )
            )
    n_params = len(in_names)
    all_names = list(in_names) + list(out_names)
    if partition_name is not None:
        all_names.append(partition_name)
    all_names = tuple(all_names)

    per_core_vals = {
        "x": [Xf[c * NIMG : (c + 1) * NIMG] for c in range(NCORES)],
        "a": [A] * NCORES,
        "xg": [
            Xf[c * NIMG : (c + 1) * NIMG, HH - 2 : HH + 2, :] for c in range(NCORES)
        ],
        "kr": [kr] * NCORES,
    }
    concat_in = [
        np.ascontiguousarray(np.concatenate(per_core_vals[n], axis=0))
        for n in in_names
    ]
    concat_zeros = [
        np.zeros((NCORES * av.shape[0], *av.shape[1:]), av.dtype) for av in out_avals
    ]

    devices = jax.devices()[:NCORES]
    mesh = Mesh(jnp_np.asarray(devices), ("core",))

    def _body(*args):
        ops = list(args)
        ins, cur = ops[:n_params], ops[n_params:]
        extra = (
            [bass2jax.partition_id_tensor()] if partition_name is not None else []
        )
        cur = bass2jax._bass_exec_p.bind(
            *ins,
            *cur,
            *extra,
            out_avals=tuple(out_avals),
            in_names=all_names,
            out_names=tuple(out_names),
            lowering_input_output_aliases=(),
            sim_require_finite=True,
            sim_require_nnan=True,
            nc=nc,
        )
        return tuple(cur)

    n_args = n_params + len(out_names)
    sharded = jax.jit(
        shard_map(
            _body,
            mesh=mesh,
            in_specs=(PartitionSpec("core"),) * n_args,
            out_specs=(PartitionSpec("core"),) * len(out_names),
            check_rep=False,
        ),
        donate_argnums=tuple(range(n_params, n_args)),
        keep_unused=True,
    )

    from jax.sharding import NamedSharding

    shard = NamedSharding(mesh, PartitionSpec("core"))
    ins_dev = [jax.device_put(a, shard) for a in concat_in]
    zeros_dev = [jax.device_put(z, shard) for z in concat_zeros]
    jax.block_until_ready(ins_dev + zeros_dev)
    # warm up (compiles + first exec); its outputs seed the timing chain
    outs = sharded(*ins_dev, *zeros_dev)
    jax.block_until_ready(outs)

    timings = {}
    for chain in chain_lens:
        best = None
        for _ in range(reps):
            cur = outs
            t0 = _time.perf_counter()
            for _ in range(chain):
                cur = sharded(*ins_dev, *cur)
            jax.block_until_ready(cur)
            dt = _time.perf_counter() - t0
            best = dt if best is None else min(best, dt)
            outs = cur
        timings[chain] = best
        print(f"chain={chain}: best wall {best * 1e3:.3f} ms "
              f"({best / chain * 1e3:.3f} ms/exec)")

    c0, c1 = min(chain_lens), max(chain_lens)
    per_exec_ns = (timings[c1] - timings[c0]) / (c1 - c0) * 1e9
    return per_exec_ns, timings


# revision 18
# speedup vs baseline: 2.5486x; 2.5486x over previous
"""Depthwise 3x3 conv (single shared 2D kernel), pad=1 stride=1.

X: (16, 64, 256, 256) f32, K: (3, 3) f32  ->  out same shape as X.

Strategy: data-parallel over the 8 NeuronCores; each core gets 128 of the
1024 (B*C) independent 256x256 images.

Per-core compute: express the H-direction 3-tap conv as a banded-matrix
matmul on the TensorEngine (contraction over the partition axis), and the
W-direction taps as free-axis offsets of the rhs access pattern, so one
PSUM bank accumulates all 9 taps in 3 matmuls. An image is split into two
128-row halves living side by side in the free axis ([128, 2, 256] tiles).
The two output rows that straddle the half boundary (127, 128) can't be
produced by a 128-partition contraction, so they are computed for all 128
images at once in a small batched DVE pass (partition axis = image index).

float32r is used for the matmuls (full-rate PE streaming vs 4x-slow fp32).
"""

import numpy as np

B, C, H, W = 16, 64, 256, 256
NCORES = 8
NIMG = (B * C) // NCORES  # 128 images per core
HH = H // 2  # 128

_PROGRAM = None


def _build_program():
    import concourse.bacc as bacc
    import concourse.mybir as mybir
    import concourse.tile as tile

    f32 = mybir.dt.float32
    f32r = mybir.dt.float32r
    mult = mybir.AluOpType.mult
    add = mybir.AluOpType.add

    nc = bacc.Bacc(
        "TRN2", target_bir_lowering=False, debug=False, num_devices=NCORES
    )
    # x/a are float32r (TF32) end-to-end: the BIR verifier requires every
    # producer feeding an FP32r matmult to itself be FP32r-typed.
    x = nc.declare_dram_parameter("x", [NIMG, H, W], f32r, isOutput=False)
    a = nc.declare_dram_parameter("a", [3, 128, 128], f32r, isOutput=False)
    # rows 126..129 of every image, host-sliced, plain f32 for the DVE pass
    xg = nc.declare_dram_parameter("xg", [NIMG, 4, W], f32, isOutput=False)
    kr = nc.declare_dram_parameter("kr", [128, 9], f32, isOutput=False)
    y = nc.declare_dram_parameter("y", [NIMG, H, W], f32, isOutput=True)

    with tile.TileContext(nc) as tc:
        with (
            tc.tile_pool(name="wts", bufs=1) as wts,
            tc.tile_pool(name="xin", bufs=4) as xin,
            tc.tile_pool(name="oub", bufs=4) as oub,
            tc.tile_pool(name="bnd", bufs=1) as bnd,
            tc.tile_pool(name="ps", bufs=4, space="PSUM") as ps,
            tc.tile_pool(name="pswarm", bufs=1, space="PSUM") as pswarm,
        ):
            # banded lhsT weights, one [128, 128] matrix per kernel column dj
            a_sb = wts.tile([128, 3, 128], f32r)
            nc.sync.dma_start(out=a_sb[:], in_=a.transpose([1, 0, 2]))
            kr_sb = wts.tile([128, 9], f32)
            nc.sync.dma_start(out=kr_sb[:], in_=kr[:])

            ar = a_sb[:]

            # Warm-up matmul so the PE observes a_sb's DMA tick here; the
            # image-loop matmuls then only wait on their own xt DMA (the
            # fused f32r weight-load instruction has few sync-wait slots).
            warm_ps = pswarm.tile([128, 2], f32)
            nc.tensor.matmul(
                warm_ps[:], ar[:, 1, :], ar[:, 1, 0:2], start=True, stop=True
            )

            # f32r matmuls stream element PAIRS: every free-dim count must be
            # even and the PSUM dst must start 8B-aligned at partition 0. So
            # keep the W-halo as zeroed pad columns (258-wide tiles) and make
            # all three matmuls full-width with rhs offset dj. Ring of NBUF
            # manually-reused tiles so the pads are memset only once.
            NBUF = 4
            xts = []
            for b in range(NBUF):
                xtb = xin.tile([128, 2, W + 2], f32r, name=f"xtb{b}")
                nc.gpsimd.memset(xtb[:].bitcast(f32), 0.0)
                xts.append(xtb)

            for i in range(NIMG):
                xt = xts[i % NBUF]
                # partition p holds image rows p (half 0) and p+128 (half 1)
                nc.sync.dma_start(
                    out=xt[:, :, 1 : W + 1],
                    in_=x[i].rearrange("(t p) w -> p t w", t=2),
                )
                pt = ps.tile([128, 2, W], f32)
                # out[m, h, w] += sum_h' A_dj[h', m] * X[h', h, w + dj - 1]
                for dj in range(3):
                    nc.tensor.matmul(
                        pt[:, :, :],
                        ar[:, dj, :],
                        xt[:, :, dj : dj + W],
                        start=(dj == 0),
                        stop=(dj == 2),
                    )
                ot = oub.tile([128, 2, W], f32)
                nc.vector.tensor_copy(ot[:], pt[:])
                # rows 0..126 from half 0; rows 129..255 from half 1
                nc.sync.dma_start(out=y[i, 0 : HH - 1, :], in_=ot[0 : HH - 1, 0, :])
                nc.sync.dma_start(out=y[i, HH + 1 : H, :], in_=ot[1:128, 1, :])

            # batched pass for output rows 127/128 of every image:
            # partition axis = image index, 9-tap MAC on the DVE.
            g = bnd.tile([128, 4, W], f32)
            nc.sync.dma_start(out=g[:], in_=xg[:])
            yb = bnd.tile([128, 2, W], f32)
            for r in range(2):  # output image rows 127 (r=0) and 128 (r=1)
                out_row = yb[:, r, :]
                taps = [(0, 1)] + [
                    (di, dj) for di in range(3) for dj in range(3) if (di, dj) != (0, 1)
                ]
                for di, dj in taps:
                    kap = kr_sb[:, 3 * di + dj : 3 * di + dj + 1]
                    if dj == 1:
                        o_sl, i_sl = out_row[:, 0:W], g[:, r + di, 0:W]
                    elif dj == 0:
                        o_sl, i_sl = out_row[:, 1:W], g[:, r + di, 0 : W - 1]
                    else:
                        o_sl, i_sl = out_row[:, 0 : W - 1], g[:, r + di, 1:W]
                    if (di, dj) == (0, 1):
                        nc.vector.tensor_scalar_mul(o_sl, i_sl, kap)
                    else:
                        nc.vector.scalar_tensor_tensor(
                            o_sl, i_sl, kap, o_sl, mult, add
                        )
            nc.sync.dma_start(out=y[:, HH - 1 : HH + 1, :], in_=yb[:])

    nc.compile()
    return nc


def _get_program():
    global _PROGRAM
    if _PROGRAM is None:
        _PROGRAM = _build_program()
    return _PROGRAM


def _banded(K):
    # A[dj][h', m] = K[h'-m+1, dj] for h' in {m-1, m, m+1} within [0, 128)
    A = np.zeros((3, 128, 128), np.float32)
    for dj in range(3):
        for di in range(3):
            v = K[di, dj]
            for m in range(128):
                hp = m + di - 1
                if 0 <= hp < 128:
                    A[dj, hp, m] = v
    return A


def _run(X, K, trace=False, trace_kwargs=None):
    from concourse.bass_utils import run_bass_kernel_spmd

    X = np.ascontiguousarray(np.asarray(X), dtype=np.float32)
    K = np.ascontiguousarray(np.asarray(K), dtype=np.float32)
    assert X.shape == (B, C, H, W) and K.shape == (3, 3)

    nc = _get_program()
    A = _banded(K)
    kr = np.ascontiguousarray(np.tile(K.reshape(1, 9), (128, 1)), np.float32)
    Xf = X.reshape(B * C, H, W)
    in_maps = [
        {
            "x": np.ascontiguousarray(Xf[c * NIMG : (c + 1) * NIMG]),
            "a": A,
            "xg": np.ascontiguousarray(
                Xf[c * NIMG : (c + 1) * NIMG, HH - 2 : HH + 2, :]
            ),
            "kr": kr,
        }
        for c in range(NCORES)
    ]
    kw = {}
    if trace:
        kw["trace"] = True
        if trace_kwargs:
            kw.update(trace_kwargs)
    res = run_bass_kernel_spmd(nc, in_maps, list(range(NCORES)), **kw)
    out = np.stack([res.results[c]["y"] for c in range(NCORES)])
    return out.reshape(B, C, H, W), res


def kernel(X, K):
    out, _ = _run(X, K)
    return out


def _bench(X, K, chain_lens=(16, 64), reps=3):
    """Estimate per-execution HW time: run `chain` sequential kernel
    executions inside one jitted shard_map call (each exec's output buffers
    feed the next), and take the wall-clock slope between chain lengths to
    cancel dispatch / RPC constants."""
    import time as _time

    import jax
    import numpy as jnp_np
    from jax.sharding import Mesh, PartitionSpec
    from jax.experimental.shard_map import shard_map

    from concourse import bass2jax

    bass2jax.install_neuronx_cc_hook()

    X = np.ascontiguousarray(np.asarray(X), dtype=np.float32)
    K = np.ascontiguousarray(np.asarray(K), dtype=np.float32)
    nc = _get_program()
    A = _banded(K)
    kr = np.ascontiguousarray(np.tile(K.reshape(1, 9), (128, 1)), np.float32)
    Xf = X.reshape(B * C, H, W)

    import concourse.mybir as mybir

    partition_name = (
        nc.partition_id_tensor.name if nc.partition_id_tensor else None
    )
    in_names, out_names, out_avals = [], [], []
    for alloc in nc.m.functions[0].allocations:
        if not isinstance(alloc, mybir.MemoryLocationSet):
            continue
        name = alloc.memorylocations[0].name
        if alloc.kind == "ExternalInput":
            if name != partition_name:
                in_names.append(name)
        elif alloc.kind == "ExternalOutput":
            out_names.append(name)
            out_avals.append(
                jax.core.ShapedArray(
                    tuple(alloc.tensor_shape), mybir.dt.np(alloc.dtype)
                )
            )
    n_params = len(in_names)
    all_names = list(in_names) + list(out_names)
    if partition_name is not None:
        all_names.append(partition_name)
    all_names = tuple(all_names)

    per_core_vals = {
        "x": [Xf[c * NIMG : (c + 1) * NIMG] for c in range(NCORES)],
        "a": [A] * NCORES,
        "xg": [
            Xf[c * NIMG : (c + 1) * NIMG, HH - 2 : HH + 2, :] for c in range(NCORES)
        ],
        "kr": [kr] * NCORES,
    }
    concat_in = [
        np.ascontiguousarray(np.concatenate(per_core_vals[n], axis=0))
        for n in in_names
    ]
    concat_zeros = [
        np.zeros((NCORES * av.shape[0], *av.shape[1:]), av.dtype) for av in out_avals
    ]

    devices = jax.devices()[:NCORES]
    mesh = Mesh(jnp_np.asarray(devices), ("core",))

    def _body(*args):
        ops = list(args)
        ins, cur = ops[:n_params], ops[n_params:]
        extra = (
            [bass2jax.partition_id_tensor()] if partition_name is not None else []
        )
        cur = bass2jax._bass_exec_p.bind(
            *ins,
            *cur,
            *extra,
            out_avals=tuple(out_avals),
            in_names=all_names,
            out_names=tuple(out_names),
            lowering_input_output_aliases=(),
            sim_require_finite=True,
            sim_require_nnan=True,
            nc=nc,
        )
        return tuple(cur)

    n_args = n_params + len(out_names)
    sharded = jax.jit(
        shard_map(
            _body,
            mesh=mesh,
            in_specs=(PartitionSpec("core"),) * n_args,
            out_specs=(PartitionSpec("core"),) * len(out_names),
            check_rep=False,
        ),
        donate_argnums=tuple(range(n_params, n_args)),
        keep_unused=True,
    )

    from jax.sharding import NamedSharding

    shard = NamedSharding(mesh, PartitionSpec("core"))
    ins_dev = [jax.device_put(a, shard) for a in concat_in]
    zeros_dev = [jax.device_put(z, shard) for z in concat_zeros]
    jax.block_until_ready(ins_dev + zeros_dev)
    # warm up (compiles + first exec); its outputs seed the timing chain
    outs = sharded(*ins_dev, *zeros_dev)
    jax.block_until_ready(outs)

    timings = {}
    for chain in chain_lens:
        best = None
        for _ in range(reps):
            cur = outs
            t0 = _time.perf_counter()
            for _ in range(chain):
                cur = sharded(*ins_dev, *cur)
            jax.block_until_ready(cur)
            dt = _time.perf_counter() - t0
            best = dt if best is None else min(best, dt)
            outs = cur
        timings[chain] = best
        print(f"chain={chain}: best wall {best * 1e3:.3f} ms "
              f"({best / chain * 1e3:.3f} ms/exec)")

    c0, c1 = min(chain_lens), max(chain_lens)
    per_exec_ns = (timings[c1] - timings[c0]) / (c1 - c0) * 1e9
    return per_exec_ns, timings


# revision 21
# speedup vs baseline: 2.6704x; 1.0478x over previous
"""Depthwise 3x3 conv (single shared 2D kernel), pad=1 stride=1.

X: (16, 64, 256, 256) f32, K: (3, 3) f32  ->  out same shape as X.

Strategy: data-parallel over the 8 NeuronCores; each core gets 128 of the
1024 (B*C) independent 256x256 images.

Per-core compute: express the H-direction 3-tap conv as a banded-matrix
matmul on the TensorEngine (contraction over the partition axis), and the
W-direction taps as free-axis offsets of the rhs access pattern, so one
PSUM bank accumulates all 9 taps in 3 matmuls. An image is split into two
128-row halves living side by side in the free axis ([128, 2, 256] tiles).
The two output rows that straddle the half boundary (127, 128) can't be
produced by a 128-partition contraction, so they are computed for all 128
images at once in a small batched DVE pass (partition axis = image index).

float32r is used for the matmuls (full-rate PE streaming vs 4x-slow fp32).
"""

import numpy as np

B, C, H, W = 16, 64, 256, 256
NCORES = 8
NIMG = (B * C) // NCORES  # 128 images per core
HH = H // 2  # 128

_PROGRAM = None


def _build_program():
    import concourse.bacc as bacc
    import concourse.mybir as mybir
    import concourse.tile as tile

    f32 = mybir.dt.float32
    f32r = mybir.dt.float32r
    mult = mybir.AluOpType.mult
    add = mybir.AluOpType.add

    nc = bacc.Bacc(
        "TRN2", target_bir_lowering=False, debug=False, num_devices=NCORES
    )
    # x/a are float32r (TF32) end-to-end: the BIR verifier requires every
    # producer feeding an FP32r matmult to itself be FP32r-typed.
    x = nc.declare_dram_parameter("x", [NIMG, H, W], f32r, isOutput=False)
    a = nc.declare_dram_parameter("a", [3, 128, 128], f32r, isOutput=False)
    # rows 126..129 of every image, host-sliced, plain f32 for the DVE pass
    xg = nc.declare_dram_parameter("xg", [NIMG, 4, W], f32, isOutput=False)
    kr = nc.declare_dram_parameter("kr", [128, 9], f32, isOutput=False)
    y = nc.declare_dram_parameter("y", [NIMG, H, W], f32, isOutput=True)

    with tile.TileContext(nc) as tc:
        with (
            tc.tile_pool(name="wts", bufs=1) as wts,
            tc.tile_pool(name="xin", bufs=4) as xin,
            tc.tile_pool(name="oub", bufs=4) as oub,
            tc.tile_pool(name="bnd", bufs=1) as bnd,
            tc.tile_pool(name="ps", bufs=2, space="PSUM") as ps,
        ):
            # banded lhsT weights, one [128, 128] matrix per kernel column dj
            a_sb = wts.tile([128, 3, 128], f32r)
            nc.sync.dma_start(out=a_sb[:], in_=a.transpose([1, 0, 2]))
            kr_sb = wts.tile([128, 9], f32)
            nc.sync.dma_start(out=kr_sb[:], in_=kr[:])

            ar = a_sb[:]

            # Warm-up matmul so the PE observes a_sb's DMA tick here; the
            # image-loop matmuls then only wait on their own xt DMA (the
            # fused f32r weight-load instruction has few sync-wait slots).
            warm_ps = ps.tile([128, 2], f32, tag="pt")
            nc.tensor.matmul(
                warm_ps[:], ar[:, 1, :], ar[:, 1, 0:2], start=True, stop=True
            )

            # f32r matmuls stream element PAIRS: every free-dim count must be
            # even and the PSUM dst must start 8B-aligned at partition 0. So
            # keep the W-halo as zeroed pad columns (258-wide tiles) and make
            # all three matmuls full-width with rhs offset dj. Ring of NBUF
            # manually-reused tiles so the pads are memset only once.
            #
            # GRP images ride in one tile so each DMA instruction moves
            # GRP*256KB: the SP/ACT sequencers and the HWDGE pay a fixed
            # ~0.6-0.8us per DMA *instruction*, which was the bottleneck at
            # one-image granularity (388 DMAs ~= 306us of SEQ occupancy).
            # Input DMAs issue on SP (nc.sync), output DMAs on ACT
            # (nc.scalar) to halve per-sequencer pressure.
            GRP = 4
            NGRP = NIMG // GRP
            NBUF = 3
            xts = []
            for b in range(NBUF):
                xtb = xin.tile([128, GRP, 2, W + 2], f32r, name=f"xtb{b}")
                nc.gpsimd.memset(xtb[:].bitcast(f32), 0.0)
                xts.append(xtb)

            for gi in range(NGRP):
                i0 = gi * GRP
                xt = xts[gi % NBUF]
                # partition p holds image rows p (half 0) and p+128 (half 1)
                nc.sync.dma_start(
                    out=xt[:, :, :, 1 : W + 1],
                    in_=x[i0 : i0 + GRP].rearrange("g (t p) w -> p g t w", t=2),
                )
                pt = ps.tile([128, GRP, 2, W], f32, tag="pt")
                for b in range(GRP):
                    # out[m, h, w] += sum_h' A_dj[h', m] * X[h', h, w + dj - 1]
                    for dj in range(3):
                        nc.tensor.matmul(
                            pt[:, b, :, :],
                            ar[:, dj, :],
                            xt[:, b, :, dj : dj + W],
                            start=(dj == 0),
                            stop=(dj == 2),
                        )
                    ot = oub.tile([128, 2, W], f32)
                    nc.vector.tensor_copy(ot[:], pt[:, b, :, :])
                    # rows 0..126 from half 0 (SP); rows 129..255 from
                    # half 1 (ACT) — split across the two HWDGE sequencers
                    nc.sync.dma_start(
                        out=y[i0 + b, 0 : HH - 1, :], in_=ot[0 : HH - 1, 0, :]
                    )
                    nc.scalar.dma_start(
                        out=y[i0 + b, HH + 1 : H, :], in_=ot[1:128, 1, :]
                    )

            # batched pass for output rows 127/128 of every image:
            # partition axis = image index, 9-tap MAC on the DVE.
            g = bnd.tile([128, 4, W], f32)
            nc.sync.dma_start(out=g[:], in_=xg[:])
            yb = bnd.tile([128, 2, W], f32)
            for r in range(2):  # output image rows 127 (r=0) and 128 (r=1)
                out_row = yb[:, r, :]
                taps = [(0, 1)] + [
                    (di, dj) for di in range(3) for dj in range(3) if (di, dj) != (0, 1)
                ]
                for di, dj in taps:
                    kap = kr_sb[:, 3 * di + dj : 3 * di + dj + 1]
                    if dj == 1:
                        o_sl, i_sl = out_row[:, 0:W], g[:, r + di, 0:W]
                    elif dj == 0:
                        o_sl, i_sl = out_row[:, 1:W], g[:, r + di, 0 : W - 1]
                    else:
                        o_sl, i_sl = out_row[:, 0 : W - 1], g[:, r + di, 1:W]
                    if (di, dj) == (0, 1):
                        nc.vector.tensor_scalar_mul(o_sl, i_sl, kap)
                    else:
                        nc.vector.scalar_tensor_tensor(
                            o_sl, i_sl, kap, o_sl, mult, add
                        )
            nc.sync.dma_start(out=y[:, HH - 1 : HH + 1, :], in_=yb[:])

    nc.compile()
    return nc


def _get_program():
    global _PROGRAM
    if _PROGRAM is None:
        _PROGRAM = _build_program()
    return _PROGRAM


def _banded(K):
    # A[dj][h', m] = K[h'-m+1, dj] for h' in {m-1, m, m+1} within [0, 128)
    A = np.zeros((3, 128, 128), np.float32)
    for dj in range(3):
        for di in range(3):
            v = K[di, dj]
            for m in range(128):
                hp = m + di - 1
                if 0 <= hp < 128:
                    A[dj, hp, m] = v
    return A


def _run(X, K, trace=False, trace_kwargs=None):
    from concourse.bass_utils import run_bass_kernel_spmd

    X = np.ascontiguousarray(np.asarray(X), dtype=np.float32)
    K = np.ascontiguousarray(np.asarray(K), dtype=np.float32)
    assert X.shape == (B, C, H, W) and K.shape == (3, 3)

    nc = _get_program()
    A = _banded(K)
    kr = np.ascontiguousarray(np.tile(K.reshape(1, 9), (128, 1)), np.float32)
    Xf = X.reshape(B * C, H, W)
    in_maps = [
        {
            "x": np.ascontiguousarray(Xf[c * NIMG : (c + 1) * NIMG]),
            "a": A,
            "xg": np.ascontiguousarray(
                Xf[c * NIMG : (c + 1) * NIMG, HH - 2 : HH + 2, :]
            ),
            "kr": kr,
        }
        for c in range(NCORES)
    ]
    kw = {}
    if trace:
        kw["trace"] = True
        if trace_kwargs:
            kw.update(trace_kwargs)
    res = run_bass_kernel_spmd(nc, in_maps, list(range(NCORES)), **kw)
    out = np.stack([res.results[c]["y"] for c in range(NCORES)])
    return out.reshape(B, C, H, W), res


def kernel(X, K):
    out, _ = _run(X, K)
    return out


def _bench(X, K, chain_lens=(16, 64), reps=3):
    """Estimate per-execution HW time: run `chain` sequential kernel
    executions inside one jitted shard_map call (each exec's output buffers
    feed the next), and take the wall-clock slope between chain lengths to
    cancel dispatch / RPC constants."""
    import time as _time

    import jax
    import numpy as jnp_np
    from jax.sharding import Mesh, PartitionSpec
    from jax.experimental.shard_map import shard_map

    from concourse import bass2jax

    bass2jax.install_neuronx_cc_hook()

    X = np.ascontiguousarray(np.asarray(X), dtype=np.float32)
    K = np.ascontiguousarray(np.asarray(K), dtype=np.float32)
    nc = _get_program()
    A = _banded(K)
    kr = np.ascontiguousarray(np.tile(K.reshape(1, 9), (128, 1)), np.float32)
    Xf = X.reshape(B * C, H, W)

    import concourse.mybir as mybir

    partition_name = (
        nc.partition_id_tensor.name if nc.partition_id_tensor else None
    )
    in_names, out_names, out_avals = [], [], []
    for alloc in nc.m.functions[0].allocations:
        if not isinstance(alloc, mybir.MemoryLocationSet):
            continue
        name = alloc.memorylocations[0].name
        if alloc.kind == "ExternalInput":
            if name != partition_name:
                in_names.append(name)
        elif alloc.kind == "ExternalOutput":
            out_names.append(name)
            out_avals.append(
                jax.core.ShapedArray(
                    tuple(alloc.tensor_shape), mybir.dt.np(alloc.dtype)
                )
            )
    n_params = len(in_names)
    all_names = list(in_names) + list(out_names)
    if partition_name is not None:
        all_names.append(partition_name)
    all_names = tuple(all_names)

    per_core_vals = {
        "x": [Xf[c * NIMG : (c + 1) * NIMG] for c in range(NCORES)],
        "a": [A] * NCORES,
        "xg": [
            Xf[c * NIMG : (c + 1) * NIMG, HH - 2 : HH + 2, :] for c in range(NCORES)
        ],
        "kr": [kr] * NCORES,
    }
    concat_in = [
        np.ascontiguousarray(np.concatenate(per_core_vals[n], axis=0))
        for n in in_names
    ]
    concat_zeros = [
        np.zeros((NCORES * av.shape[0], *av.shape[1:]), av.dtype) for av in out_avals
    ]

    devices = jax.devices()[:NCORES]
    mesh = Mesh(jnp_np.asarray(devices), ("core",))

    def _body(*args):
        ops = list(args)
        ins, cur = ops[:n_params], ops[n_params:]
        extra = (
            [bass2jax.partition_id_tensor()] if partition_name is not None else []
        )
        cur = bass2jax._bass_exec_p.bind(
            *ins,
            *cur,
            *extra,
            out_avals=tuple(out_avals),
            in_names=all_names,
            out_names=tuple(out_names),
            lowering_input_output_aliases=(),
            sim_require_finite=True,
            sim_require_nnan=True,
            nc=nc,
        )
        return tuple(cur)

    n_args = n_params + len(out_names)
    sharded = jax.jit(
        shard_map(
            _body,
            mesh=mesh,
            in_specs=(PartitionSpec("core"),) * n_args,
            out_specs=(PartitionSpec("core"),) * len(out_names),
            check_rep=False,
        ),
        donate_argnums=tuple(range(n_params, n_args)),
        keep_unused=True,
    )

    from jax.sharding import NamedSharding

    shard = NamedSharding(mesh, PartitionSpec("core"))
    ins_dev = [jax.device_put(a, shard) for a in concat_in]
    zeros_dev = [jax.device_put(z, shard) for z in concat_zeros]
    jax.block_until_ready(ins_dev + zeros_dev)
    # warm up (compiles + first exec); its outputs seed the timing chain
    outs = sharded(*ins_dev, *zeros_dev)
    jax.block_until_ready(outs)

    timings = {}
    for chain in chain_lens:
        best = None
        for _ in range(reps):
            cur = outs
            t0 = _time.perf_counter()
            for _ in range(chain):
                cur = sharded(*ins_dev, *cur)
            jax.block_until_ready(cur)
            dt = _time.perf_counter() - t0
            best = dt if best is None else min(best, dt)
            outs = cur
        timings[chain] = best
        print(f"chain={chain}: best wall {best * 1e3:.3f} ms "
              f"({best / chain * 1e3:.3f} ms/exec)")

    c0, c1 = min(chain_lens), max(chain_lens)
    per_exec_ns = (timings[c1] - timings[c0]) / (c1 - c0) * 1e9
    return per_exec_ns, timings
